# revision 12
# baseline (speedup 1.0000x reference)
"""Trainium2 Bass kernel for nn_Attention_7945689497706.

Distribution: data-parallel over batch, 2 batch elements per core, weights
replicated, no collectives.

v2 design (vs baseline):
  - Host folds (gamma+1) and dh^-0.5 into bf16 weights; x shipped bf16
    (halves input DMA, no on-chip weight prep).
  - Keys reordered per head as [pixels 0:1020 | mem(4) | pixels 1020:1024]:
    mem_kv rides chunk 7 rows 124:128 (DMA'd into place), the 4 leftover
    pixels form a tiny 9th chunk packed 4-pairs-per-psum via col tiling.
  - sim matmuls of a head PAIR run concurrently on disjoint PE row groups
    (even head rows 0:64, odd head rows 64:128) -> 2x sim throughput.
  - exp at [128, 1024] grain covering both heads of the pair.
  - softmax denominators via ones-column of v; per (pair, h2) a single K=2
    selector matmul broadcasts both heads' denominators, DVE reciprocal +
    psum-direct muls assemble attn.
  - Flat loop over 8 (batch, pair) units; qkv proj of batch 1 and out proj
    of batch 0 are spread as PE filler inside the loop.
"""

import numpy as np

import concourse.bass as bass
import concourse.mybir as mybir
import concourse.tile as tile
from concourse import bacc
from concourse.bass_utils import run_bass_kernel_spmd

F32 = mybir.dt.float32
F32R = mybir.dt.float32r
BF16 = mybir.dt.bfloat16
AF = mybir.ActivationFunctionType

NCORES = 8
B = 16
C = 512
N = 1024          # pixels = 32*32
HEADS = 8
DH = 64
NMEM = 4
PB = B // NCORES  # batch elements per core
CT = C // 128     # channel partition-tiles
NPAIR = 4         # head pairs
VW = DH + 1       # per head [v | ones]
KC = 1028         # keys per head: 1020 px + 4 mem + 4 px


def _build():
    nc = bacc.Bacc()
    x_ext = nc.declare_dram_parameter("x", [PB, C, N], BF16, isOutput=False)
    wqkv_ext = nc.declare_dram_parameter("wqkv", [C, 3 * C], BF16, isOutput=False)
    wot_ext = nc.declare_dram_parameter("wot", [C, C], BF16, isOutput=False)
    memk_ext = nc.declare_dram_parameter("memk", [128, HEADS, NMEM], BF16, isOutput=False)
    memv_ext = nc.declare_dram_parameter("memv", [NMEM, HEADS * VW], BF16, isOutput=False)
    out_ext = nc.declare_dram_parameter("out", [PB, C, N], F32, isOutput=True)

    with tile.TileContext(nc) as tc:
        with (
            tc.tile_pool(name="const", bufs=1) as const,
            tc.tile_pool(name="data", bufs=2) as data,
            tc.tile_pool(name="pp", bufs=4) as pp,
            tc.tile_pool(name="rp", bufs=2) as rp,
            tc.tile_pool(name="dvp", bufs=2) as dvp,
            tc.tile_pool(name="obp", bufs=2) as obp,
            tc.tile_pool(name="sim_ps", bufs=2, space="PSUM") as sim_ps,
            tc.tile_pool(name="av_ps", bufs=1, space="PSUM") as av_ps,
            tc.tile_pool(name="proj_ps", bufs=2, space="PSUM") as proj_ps,
        ):
            # ---------------- constants / big tiles ----------------
            wqkv = const.tile([128, CT, 3 * C], BF16, tag="wqkv")
            wo = const.tile([128, CT, C], BF16, tag="wo")
            ones128 = const.tile([128, 128], BF16, tag="ones128")
            ones1 = const.tile([128, 64], F32R, tag="ones1")

            def btiles(nm, shape, dtype):
                return [const.tile(shape, dtype, tag=f"{nm}{b}", name=f"{nm}{b}")
                        for b in range(PB)]

            xbs = btiles("xb", [128, CT, N], BF16)
            xns = btiles("xn", [128, CT, N], BF16)
            qTs = btiles("qT", [128, NPAIR, N], BF16)
            kTps = btiles("kTp", [128, HEADS, KC], BF16)
            vexts = btiles("vext", [128, 8, HEADS * VW], BF16)
            vext9s = btiles("v9", [128, HEADS * VW], BF16)
            p9s = btiles("p9", [128, 2, N], BF16)
            attns = btiles("attn", [128, CT, N], BF16)
            snorms = btiles("snorm", [128, N], F32)

            # ---------------- DMA plan ----------------
            # x0 first (norm0 gates everything), then pair-0 weight columns,
            # then x1, then the rest of the weights.
            for t in range(CT):
                nc.sync.dma_start(out=xbs[0][:, t, :], in_=x_ext[0, t * 128:(t + 1) * 128, :])
            for t in range(CT):
                nc.scalar.dma_start(out=wqkv[:, t, C:C + 128],
                                    in_=wqkv_ext[t * 128:(t + 1) * 128, C:C + 128])
                nc.scalar.dma_start(out=wqkv[:, t, 0:128],
                                    in_=wqkv_ext[t * 128:(t + 1) * 128, 0:128])
            for t in range(CT):
                nc.sync.dma_start(out=xbs[1][:, t, :], in_=x_ext[1, t * 128:(t + 1) * 128, :])
            for p in range(1, NPAIR):
                for t in range(CT):
                    nc.scalar.dma_start(
                        out=wqkv[:, t, C + 128 * p:C + 128 * (p + 1)],
                        in_=wqkv_ext[t * 128:(t + 1) * 128, C + 128 * p:C + 128 * (p + 1)])
                    nc.scalar.dma_start(
                        out=wqkv[:, t, 128 * p:128 * (p + 1)],
                        in_=wqkv_ext[t * 128:(t + 1) * 128, 128 * p:128 * (p + 1)])
            for t in range(CT):
                nc.scalar.dma_start(out=wqkv[:, t, 2 * C:3 * C],
                                    in_=wqkv_ext[t * 128:(t + 1) * 128, 2 * C:3 * C])
            for t in range(CT):
                nc.scalar.dma_start(out=wo[:, t, :], in_=wot_ext[t * 128:(t + 1) * 128, :])
            # mem K straight into place (kTp cols 1020:1024; disjoint from the
            # kchain writes). mem V is DMA'd inside vchain(b, 7) for ordering.
            for b in range(PB):
                nc.sync.dma_start(out=kTps[b][:, :, 1020:1024], in_=memk_ext[:, :, :])

            nc.vector.memset(ones128, 1.0)
            nc.vector.memset(ones1.bitcast(F32), 1.0)
            # ones columns of vext, full-partition memsets (chunk-7 mem rows are
            # later overwritten by the memv DMA, which also carries 1.0 there)
            for b in range(PB):
                oc = vexts[b][:, :, :].rearrange("p j (h c) -> p j h c", c=VW)[:, :, :, DH:DH + 1]
                nc.gpsimd.memset(oc, 1.0)

            # ---------------- norm ----------------
            def norm(b):
                xb = xbs[b]
                xsq = data.tile([128, CT, N], BF16, tag="xsq", bufs=1)
                for t in range(CT):
                    nc.vector.tensor_mul(out=xsq[:, t, :], in0=xb[:, t, :], in1=xb[:, t, :])
                ss = sim_ps.tile([128, N], F32, tag="sim")
                for h2 in range(2):
                    for t in range(CT):
                        nc.tensor.matmul(ss[:, h2 * 512:(h2 + 1) * 512], ones128,
                                         xsq[:, t, h2 * 512:(h2 + 1) * 512],
                                         start=(t == 0), stop=(t == CT - 1))
                sroot = data.tile([128, N], F32, tag="sroot")
                nc.scalar.activation(out=sroot, in_=ss, func=AF.Sqrt, scale=1.0 / C)
                nc.vector.reciprocal_approx_fast(out=snorms[b], in_=sroot)
                for t in range(CT):
                    nc.vector.tensor_mul(out=xns[b][:, t, :], in0=xb[:, t, :], in1=snorms[b])

            # ---------------- projection chains ----------------
            def qchain(b, p, h2):
                ps = proj_ps.tile([128, 512], F32, tag="proj")
                for t in range(CT):
                    nc.tensor.matmul(ps, wqkv[:, t, 128 * p:128 * (p + 1)],
                                     xns[b][:, t, h2 * 512:(h2 + 1) * 512],
                                     start=(t == 0), stop=(t == CT - 1))
                nc.vector.tensor_copy(out=qTs[b][:, p, h2 * 512:(h2 + 1) * 512], in_=ps)

            def kchain(b, p, h2):
                ps = proj_ps.tile([128, 512], F32, tag="proj")
                for t in range(CT):
                    nc.tensor.matmul(ps, wqkv[:, t, C + 128 * p:C + 128 * (p + 1)],
                                     xns[b][:, t, h2 * 512:(h2 + 1) * 512],
                                     start=(t == 0), stop=(t == CT - 1))
                # kTp cols = pixel index for px<1020, 1024+ for the leftover 4
                kTp = kTps[b]
                h0, h1 = 2 * p, 2 * p + 1
                if h2 == 0:
                    nc.vector.tensor_copy(out=kTp[0:64, h0, 0:512], in_=ps[0:64, :])
                    nc.vector.tensor_copy(out=kTp[64:128, h1, 0:512], in_=ps[64:128, :])
                else:
                    nc.vector.tensor_copy(out=kTp[0:64, h0, 512:1020], in_=ps[0:64, 0:508])
                    nc.vector.tensor_copy(out=kTp[64:128, h1, 512:1020], in_=ps[64:128, 0:508])
                    nc.vector.tensor_copy(out=kTp[0:64, h0, 1024:1028], in_=ps[0:64, 508:512])
                    nc.vector.tensor_copy(out=kTp[64:128, h1, 1024:1028], in_=ps[64:128, 508:512])

            def vchain(b, ic):
                ps = proj_ps.tile([128, 512], F32, tag="proj")
                for t in range(CT):
                    nc.tensor.matmul(ps, xns[b][:, t, ic * 128:(ic + 1) * 128],
                                     wqkv[:, t, 2 * C:3 * C],
                                     start=(t == 0), stop=(t == CT - 1))
                ps_h = ps[:, :].rearrange("p (h c) -> p h c", c=DH)
                vdst = vexts[b][:, ic, :].rearrange("p (h c) -> p h c", c=VW)[:, :, 0:DH]
                nc.vector.tensor_copy(out=vdst, in_=ps_h)
                if ic == 7:
                    # rows 124:128 now hold the leftover-pixel v (+ones col from
                    # the memset): relocate each pair's 130-col block to its
                    # 32p row slot in vext9, then overwrite rows 124:128 with
                    # mem V (the DMA also carries its ones).
                    for p in range(NPAIR):
                        nc.sync.dma_start(
                            out=vext9s[b][32 * p:32 * p + NMEM, 2 * p * VW:(2 * p + 2) * VW],
                            in_=vexts[b][124:128, 7, 2 * p * VW:(2 * p + 2) * VW])
                    nc.sync.dma_start(out=vexts[b][124:128, 7, :], in_=memv_ext[:, :])

            def ochain(b, mc, h2):
                ps = proj_ps.tile([128, 512], F32, tag="proj")
                for t in range(CT):
                    nc.tensor.matmul(ps, wo[:, t, mc * 128:(mc + 1) * 128],
                                     attns[b][:, t, h2 * 512:(h2 + 1) * 512],
                                     start=(t == 0), stop=(t == CT - 1))
                ob = obp.tile([128, 512], F32, tag="ob")
                nc.vector.tensor_copy(out=ob, in_=ps)
                nc.sync.dma_start(
                    out=out_ext[b, mc * 128:(mc + 1) * 128, h2 * 512:(h2 + 1) * 512],
                    in_=ob)

            # ---------------- chunk-8 (leftover 4 pixels), all pairs packed ----
            def sim9(b):
                kTp, qT = kTps[b], qTs[b]
                for h2 in range(2):
                    st9e = proj_ps.tile([128, 512], F32, tag="proj")
                    for p in range(NPAIR):
                        nc.tensor.matmul(st9e[32 * p:32 * p + NMEM, :],
                                         kTp[0:64, 2 * p, 1024:1028],
                                         qT[0:64, p, h2 * 512:(h2 + 1) * 512],
                                         start=True, stop=True,
                                         tile_position=(0, 32 * p))
                    nc.scalar.activation(out=p9s[b][:, h2, 0:512], in_=st9e, func=AF.Exp)
                    st9o = proj_ps.tile([128, 512], F32, tag="proj")
                    for p in range(NPAIR):
                        nc.tensor.matmul(st9o[32 * p:32 * p + NMEM, :],
                                         kTp[64:128, 2 * p + 1, 1024:1028],
                                         qT[64:128, p, h2 * 512:(h2 + 1) * 512],
                                         start=True, stop=True,
                                         tile_position=(64, 32 * p))
                    nc.scalar.activation(out=p9s[b][:, h2, 512:1024], in_=st9o, func=AF.Exp)

            # ---------------- one attention unit: (batch, pair) ----------------
            def pair_attn(b, p, filler):
                kTp, qT, vext = kTps[b], qTs[b], vexts[b]
                h0, h1 = 2 * p, 2 * p + 1
                c0, c1 = h0 * VW, h1 * VW
                for h2 in range(2):
                    avE = av_ps.tile([128, 512], F32, tag="avE")
                    avO = av_ps.tile([128, 512], F32, tag="avO")
                    for jc in range(8):
                        st = sim_ps.tile([128, N], F32, tag="sim")
                        nc.tensor.matmul(st[:, 0:512],
                                         kTp[0:64, h0, jc * 128:(jc + 1) * 128],
                                         qT[0:64, p, h2 * 512:(h2 + 1) * 512],
                                         start=True, stop=True)
                        nc.tensor.matmul(st[:, 512:1024],
                                         kTp[64:128, h1, jc * 128:(jc + 1) * 128],
                                         qT[64:128, p, h2 * 512:(h2 + 1) * 512],
                                         start=True, stop=True)
                        pt = pp.tile([128, N], BF16, tag="p")
                        nc.scalar.activation(out=pt, in_=st, func=AF.Exp)
                        nc.tensor.matmul(avE[0:VW, :], vext[:, jc, c0:c0 + VW], pt[:, 0:512],
                                         start=(jc == 0), stop=False)
                        nc.tensor.matmul(avO[0:VW, :], vext[:, jc, c1:c1 + VW], pt[:, 512:1024],
                                         start=(jc == 0), stop=False)
                        if filler is not None:
                            filler(b, p, h2, jc)
                    # leftover-pixel chunk closes the accumulation
                    nc.tensor.matmul(avE[0:VW, :],
                                     vext9s[b][32 * p:32 * p + NMEM, c0:c0 + VW],
                                     p9s[b][32 * p:32 * p + NMEM, h2, 0:512],
                                     start=False, stop=True,
                                     tile_position=(32 * p, 0))
                    nc.tensor.matmul(avO[0:VW, :],
                                     vext9s[b][32 * p:32 * p + NMEM, c1:c1 + VW],
                                     p9s[b][32 * p:32 * p + NMEM, h2, 512:1024],
                                     start=False, stop=True,
                                     tile_position=(32 * p, 0))
                    # denominators -> broadcast -> reciprocal -> attn
                    with tc.high_priority(offset=96):
                        dv = dvp.tile([128, 1024], F32R, tag="dv")
                        nc.vector.tensor_copy(out=dv[64:65, 0:512], in_=avE[64:65, :])
                        nc.vector.tensor_copy(out=dv[64:65, 512:1024], in_=avO[64:65, :])
                        bcpE = proj_ps.tile([128, 512], F32, tag="proj")
                        nc.tensor.matmul(bcpE[0:64, :], ones1[64:65, :], dv[64:65, 0:512],
                                         start=True, stop=True)
                        bcpO = proj_ps.tile([128, 512], F32, tag="proj")
                        nc.tensor.matmul(bcpO[0:64, :], ones1[64:65, :], dv[64:65, 512:1024],
                                         start=True, stop=True)
                        rcpE = rp.tile([64, 512], F32, tag="rcpE")
                        nc.vector.reciprocal_approx_fast(out=rcpE, in_=bcpE[0:64, :])
                        rcpO = rp.tile([64, 512], F32, tag="rcpO")
                        nc.vector.reciprocal_approx_fast(out=rcpO, in_=bcpO[0:64, :])
                        nc.vector.tensor_mul(
                            out=attns[b][0:64, p, h2 * 512:(h2 + 1) * 512],
                            in0=avE[0:64, :], in1=rcpE)
                        nc.vector.tensor_mul(
                            out=attns[b][64:128, p, h2 * 512:(h2 + 1) * 512],
                            in0=avO[0:64, :], in1=rcpO)

            # ---------------- schedule ----------------
            norm(0)
            norm(1)
            for p in range(NPAIR):
                for h2 in range(2):
                    kchain(0, p, h2)
                    qchain(0, p, h2)
            vchain(0, 7)
            for ic in range(7):
                vchain(0, ic)
            sim9(0)

            # filler queues: batch-1 qkv, its chunk-8 sims, batch-0 out proj
            fills = []
            for p in range(NPAIR):
                for h2 in range(2):
                    fills.append(lambda p=p, h2=h2: kchain(1, p, h2))
                    fills.append(lambda p=p, h2=h2: qchain(1, p, h2))
            fills.append(lambda: vchain(1, 7))
            fills.append(lambda: sim9(1))
            for ic in range(7):
                fills.append(lambda ic=ic: vchain(1, ic))
            for mc in range(CT):
                for h2 in range(2):
                    fills.append(lambda mc=mc, h2=h2: ochain(0, mc, h2))
            fcount = [0]

            def filler(b, p, h2, jc):
                unit = (b * NPAIR + p) * 16 + h2 * 8 + jc
                want = min(len(fills), (unit * 36) // 96 + 1)
                while fcount[0] < want:
                    fills[fcount[0]]()
                    fcount[0] += 1

            for b in range(PB):
                for p in range(NPAIR):
                    pair_attn(b, p, filler)
            while fcount[0] < len(fills):
                fills[fcount[0]]()
                fcount[0] += 1
            for mc in range(CT):
                for h2 in range(2):
                    ochain(1, mc, h2)
    nc.compile()
    return nc


_NC_CACHE = []


def _prep_inputs(x, gamma, mem_kv, w_qkv, w_out):
    b, c, hh, ww = x.shape
    n = hh * ww
    xs = x.reshape(b, c, n)

    g1 = gamma + 1.0
    wq = w_qkv[0:C] * (DH ** -0.5)
    wkv = w_qkv[C:]
    wqkv_eff = np.concatenate([wq, wkv], axis=0) * g1[None, :]
    wqkvt = np.ascontiguousarray(wqkv_eff.T)       # [c, 3c]
    wot = np.ascontiguousarray(w_out.T)            # [c, c]

    # memk: [128, heads, 4] - even head rows 0:64, odd head rows 64:128
    memk = np.zeros((128, HEADS, NMEM), np.float32)
    for h in range(HEADS):
        r0 = 64 * (h % 2)
        memk[r0:r0 + DH, h] = mem_kv[0, h].T
    # memv: [4, heads*(dh+1)] - v then the ones column
    memv = np.zeros((NMEM, HEADS * VW), np.float32)
    for h in range(HEADS):
        memv[:, h * VW:h * VW + DH] = mem_kv[1, h]
        memv[:, h * VW + DH] = 1.0

    import jax.numpy as jnp

    def bf(a):
        return np.asarray(jnp.asarray(a, dtype=jnp.bfloat16))

    return xs, bf(xs), bf(wqkvt), bf(wot), bf(memk), bf(memv)


def kernel(x, gamma, mem_kv, w_qkv, w_out, _trace=False):
    x = np.asarray(x, dtype=np.float32)
    gamma = np.asarray(gamma, dtype=np.float32)
    mem_kv = np.asarray(mem_kv, dtype=np.float32)
    w_qkv = np.asarray(w_qkv, dtype=np.float32)
    w_out = np.asarray(w_out, dtype=np.float32)

    b, c, hh, ww = x.shape
    xs, xbf, wqkvt, wot, memk, memv = _prep_inputs(x, gamma, mem_kv, w_qkv, w_out)

    if not _NC_CACHE:
        _NC_CACHE.append(_build())
    nc = _NC_CACHE[0]

    in_maps = []
    for core in range(NCORES):
        in_maps.append({
            "x": np.ascontiguousarray(xbf[core * PB:(core + 1) * PB]),
            "wqkv": wqkvt,
            "wot": wot,
            "memk": memk,
            "memv": memv,
        })
    res = run_bass_kernel_spmd(nc, in_maps, core_ids=list(range(NCORES)), trace=_trace)
    out = np.concatenate([res.results[core]["out"] for core in range(NCORES)], axis=0)
    kernel.last_result = res
    return out.reshape(b, c, hh, ww)


# revision 13
# speedup vs baseline: 1.0559x; 1.0559x over previous
"""Trainium2 Bass kernel for nn_Attention_7945689497706.

Distribution: data-parallel over batch, 2 batch elements per core, weights
replicated, no collectives.

v2 design (vs baseline):
  - Host folds (gamma+1) and dh^-0.5 into bf16 weights; x shipped bf16
    (halves input DMA, no on-chip weight prep).
  - Keys reordered per head as [pixels 0:1020 | mem(4) | pixels 1020:1024]:
    mem_kv rides chunk 7 rows 124:128 (DMA'd into place), the 4 leftover
    pixels form a tiny 9th chunk packed 4-pairs-per-psum via col tiling.
  - sim matmuls of a head PAIR run concurrently on disjoint PE row groups
    (even head rows 0:64, odd head rows 64:128) -> 2x sim throughput.
  - exp at [128, 1024] grain covering both heads of the pair.
  - softmax denominators via ones-column of v; per (pair, h2) a single K=2
    selector matmul broadcasts both heads' denominators, DVE reciprocal +
    psum-direct muls assemble attn.
  - Flat loop over 8 (batch, pair) units; qkv proj of batch 1 and out proj
    of batch 0 are spread as PE filler inside the loop.
"""

import numpy as np

import concourse.bass as bass
import concourse.mybir as mybir
import concourse.tile as tile
from concourse import bacc
from concourse.bass_utils import run_bass_kernel_spmd

F32 = mybir.dt.float32
F32R = mybir.dt.float32r
BF16 = mybir.dt.bfloat16
AF = mybir.ActivationFunctionType

NCORES = 8
B = 16
C = 512
N = 1024          # pixels = 32*32
HEADS = 8
DH = 64
NMEM = 4
PB = B // NCORES  # batch elements per core
CT = C // 128     # channel partition-tiles
NPAIR = 4         # head pairs
VW = DH + 1       # per head [v | ones]
KC = 1028         # keys per head: 1020 px + 4 mem + 4 px


def _build():
    nc = bacc.Bacc()
    x_ext = nc.declare_dram_parameter("x", [PB, C, N], BF16, isOutput=False)
    wqkv_ext = nc.declare_dram_parameter("wqkv", [C, 3 * C], BF16, isOutput=False)
    wot_ext = nc.declare_dram_parameter("wot", [C, C], BF16, isOutput=False)
    memk_ext = nc.declare_dram_parameter("memk", [128, HEADS, NMEM], BF16, isOutput=False)
    memv_ext = nc.declare_dram_parameter("memv", [NMEM, HEADS * VW], BF16, isOutput=False)
    out_ext = nc.declare_dram_parameter("out", [PB, C, N], F32, isOutput=True)

    with tile.TileContext(nc) as tc:
        with (
            tc.tile_pool(name="const", bufs=1) as const,
            tc.tile_pool(name="data", bufs=2) as data,
            tc.tile_pool(name="pp", bufs=4) as pp,
            tc.tile_pool(name="rp", bufs=2) as rp,
            tc.tile_pool(name="dvp", bufs=2) as dvp,
            tc.tile_pool(name="obp", bufs=2) as obp,
            tc.tile_pool(name="sim_ps", bufs=2, space="PSUM") as sim_ps,
            tc.tile_pool(name="av_ps", bufs=1, space="PSUM") as av_ps,
            tc.tile_pool(name="proj_ps", bufs=2, space="PSUM") as proj_ps,
        ):
            # ---------------- constants / big tiles ----------------
            wqkv = const.tile([128, CT, 3 * C], BF16, tag="wqkv")
            wo = const.tile([128, CT, C], BF16, tag="wo")
            ones128 = const.tile([128, 128], BF16, tag="ones128")
            ones1 = const.tile([128, 64], F32R, tag="ones1")

            def btiles(nm, shape, dtype):
                return [const.tile(shape, dtype, tag=f"{nm}{b}", name=f"{nm}{b}")
                        for b in range(PB)]

            xbs = btiles("xb", [128, CT, N], BF16)
            xns = btiles("xn", [128, CT, N], BF16)
            qTs = btiles("qT", [128, NPAIR, N], BF16)
            kTps = btiles("kTp", [128, HEADS, KC], BF16)
            vexts = btiles("vext", [128, 8, HEADS * VW], BF16)
            vext9s = btiles("v9", [128, HEADS * VW], BF16)
            p9s = btiles("p9", [128, 2, N], BF16)
            attns = btiles("attn", [128, CT, N], BF16)
            snorms = btiles("snorm", [128, N], F32)

            # ---------------- DMA plan ----------------
            # Few, large transfers (descriptor issue costs ~600ns of engine
            # time each). x on the sync queue; weights on the gpsimd queue so
            # the scalar engine stays free for activations.
            for t in range(CT):
                nc.sync.dma_start(out=xbs[0][:, t, :], in_=x_ext[0, t * 128:(t + 1) * 128, :])
            for t in range(CT):
                nc.gpsimd.dma_start(out=wqkv[:, t, :],
                                    in_=wqkv_ext[t * 128:(t + 1) * 128, :])
            for t in range(CT):
                nc.sync.dma_start(out=xbs[1][:, t, :], in_=x_ext[1, t * 128:(t + 1) * 128, :])
            for t in range(CT):
                nc.gpsimd.dma_start(out=wo[:, t, :], in_=wot_ext[t * 128:(t + 1) * 128, :])
            # mem K straight into place (kTp cols 1020:1024; disjoint from the
            # kchain writes). mem V is DMA'd inside vchain(b, 7) for ordering.
            for b in range(PB):
                nc.gpsimd.dma_start(out=kTps[b][:, :, 1020:1024], in_=memk_ext[:, :, :])

            nc.vector.memset(ones128, 1.0)
            nc.vector.memset(ones1.bitcast(F32), 1.0)
            # ones columns of vext, full-partition memsets (chunk-7 mem rows are
            # later overwritten by the memv DMA, which also carries 1.0 there)
            for b in range(PB):
                oc = vexts[b][:, :, :].rearrange("p j (h c) -> p j h c", c=VW)[:, :, :, DH:DH + 1]
                nc.gpsimd.memset(oc, 1.0)

            # ---------------- norm ----------------
            def norm(b):
                xb = xbs[b]
                xsq = data.tile([128, CT, N], BF16, tag="xsq", bufs=1)
                for t in range(CT):
                    nc.vector.tensor_mul(out=xsq[:, t, :], in0=xb[:, t, :], in1=xb[:, t, :])
                ss = sim_ps.tile([128, N], F32, tag="sim")
                for h2 in range(2):
                    for t in range(CT):
                        nc.tensor.matmul(ss[:, h2 * 512:(h2 + 1) * 512], ones128,
                                         xsq[:, t, h2 * 512:(h2 + 1) * 512],
                                         start=(t == 0), stop=(t == CT - 1))
                sroot = data.tile([128, N], F32, tag="sroot")
                nc.scalar.activation(out=sroot, in_=ss, func=AF.Sqrt, scale=1.0 / C)
                nc.vector.reciprocal_approx_fast(out=snorms[b], in_=sroot)
                for t in range(CT):
                    nc.vector.tensor_mul(out=xns[b][:, t, :], in0=xb[:, t, :], in1=snorms[b])

            # ---------------- projection chains ----------------
            def qchain(b, p, h2):
                ps = proj_ps.tile([128, 512], F32, tag="proj")
                for t in range(CT):
                    nc.tensor.matmul(ps, wqkv[:, t, 128 * p:128 * (p + 1)],
                                     xns[b][:, t, h2 * 512:(h2 + 1) * 512],
                                     start=(t == 0), stop=(t == CT - 1))
                nc.vector.tensor_copy(out=qTs[b][:, p, h2 * 512:(h2 + 1) * 512], in_=ps)

            def kchain(b, p, h2):
                ps = proj_ps.tile([128, 512], F32, tag="proj")
                for t in range(CT):
                    nc.tensor.matmul(ps, wqkv[:, t, C + 128 * p:C + 128 * (p + 1)],
                                     xns[b][:, t, h2 * 512:(h2 + 1) * 512],
                                     start=(t == 0), stop=(t == CT - 1))
                # kTp cols = pixel index for px<1020, 1024+ for the leftover 4
                kTp = kTps[b]
                h0, h1 = 2 * p, 2 * p + 1
                if h2 == 0:
                    nc.vector.tensor_copy(out=kTp[0:64, h0, 0:512], in_=ps[0:64, :])
                    nc.vector.tensor_copy(out=kTp[64:128, h1, 0:512], in_=ps[64:128, :])
                else:
                    nc.vector.tensor_copy(out=kTp[0:64, h0, 512:1020], in_=ps[0:64, 0:508])
                    nc.vector.tensor_copy(out=kTp[64:128, h1, 512:1020], in_=ps[64:128, 0:508])
                    nc.vector.tensor_copy(out=kTp[0:64, h0, 1024:1028], in_=ps[0:64, 508:512])
                    nc.vector.tensor_copy(out=kTp[64:128, h1, 1024:1028], in_=ps[64:128, 508:512])

            def vchain(b, ic):
                ps = proj_ps.tile([128, 512], F32, tag="proj")
                for t in range(CT):
                    nc.tensor.matmul(ps, xns[b][:, t, ic * 128:(ic + 1) * 128],
                                     wqkv[:, t, 2 * C:3 * C],
                                     start=(t == 0), stop=(t == CT - 1))
                ps_h = ps[:, :].rearrange("p (h c) -> p h c", c=DH)
                vdst = vexts[b][:, ic, :].rearrange("p (h c) -> p h c", c=VW)[:, :, 0:DH]
                nc.vector.tensor_copy(out=vdst, in_=ps_h)
                if ic == 7:
                    # rows 124:128 now hold the leftover-pixel v (+ones col from
                    # the memset): relocate each pair's 130-col block to its
                    # 32p row slot in vext9, then overwrite rows 124:128 with
                    # mem V (the DMA also carries its ones).
                    for p in range(NPAIR):
                        nc.sync.dma_start(
                            out=vext9s[b][32 * p:32 * p + NMEM, 2 * p * VW:(2 * p + 2) * VW],
                            in_=vexts[b][124:128, 7, 2 * p * VW:(2 * p + 2) * VW])
                    nc.sync.dma_start(out=vexts[b][124:128, 7, :], in_=memv_ext[:, :])

            def ochain(b, mc, h2):
                ps = proj_ps.tile([128, 512], F32, tag="proj")
                for t in range(CT):
                    nc.tensor.matmul(ps, wo[:, t, mc * 128:(mc + 1) * 128],
                                     attns[b][:, t, h2 * 512:(h2 + 1) * 512],
                                     start=(t == 0), stop=(t == CT - 1))
                ob = obp.tile([128, 512], F32, tag="ob")
                nc.vector.tensor_copy(out=ob, in_=ps)
                nc.sync.dma_start(
                    out=out_ext[b, mc * 128:(mc + 1) * 128, h2 * 512:(h2 + 1) * 512],
                    in_=ob)

            # ---------------- chunk-8 (leftover 4 pixels), all pairs packed ----
            def sim9(b):
                kTp, qT = kTps[b], qTs[b]
                for h2 in range(2):
                    st9e = proj_ps.tile([128, 512], F32, tag="proj")
                    for p in range(NPAIR):
                        nc.tensor.matmul(st9e[32 * p:32 * p + NMEM, :],
                                         kTp[0:64, 2 * p, 1024:1028],
                                         qT[0:64, p, h2 * 512:(h2 + 1) * 512],
                                         start=True, stop=True,
                                         tile_position=(0, 32 * p))
                    nc.scalar.activation(out=p9s[b][:, h2, 0:512], in_=st9e, func=AF.Exp)
                    st9o = proj_ps.tile([128, 512], F32, tag="proj")
                    for p in range(NPAIR):
                        nc.tensor.matmul(st9o[32 * p:32 * p + NMEM, :],
                                         kTp[64:128, 2 * p + 1, 1024:1028],
                                         qT[64:128, p, h2 * 512:(h2 + 1) * 512],
                                         start=True, stop=True,
                                         tile_position=(64, 32 * p))
                    nc.scalar.activation(out=p9s[b][:, h2, 512:1024], in_=st9o, func=AF.Exp)

            # ---------------- one attention unit: (batch, pair) ----------------
            def pair_attn(b, p, filler):
                kTp, qT, vext = kTps[b], qTs[b], vexts[b]
                h0, h1 = 2 * p, 2 * p + 1
                c0, c1 = h0 * VW, h1 * VW
                for h2 in range(2):
                    avE = av_ps.tile([128, 512], F32, tag="avE")
                    avO = av_ps.tile([128, 512], F32, tag="avO")
                    for jc in range(8):
                        st = sim_ps.tile([128, N], F32, tag="sim")
                        nc.tensor.matmul(st[:, 0:512],
                                         kTp[0:64, h0, jc * 128:(jc + 1) * 128],
                                         qT[0:64, p, h2 * 512:(h2 + 1) * 512],
                                         start=True, stop=True)
                        nc.tensor.matmul(st[:, 512:1024],
                                         kTp[64:128, h1, jc * 128:(jc + 1) * 128],
                                         qT[64:128, p, h2 * 512:(h2 + 1) * 512],
                                         start=True, stop=True)
                        pt = pp.tile([128, N], BF16, tag="p")
                        nc.scalar.activation(out=pt, in_=st, func=AF.Exp)
                        nc.tensor.matmul(avE[0:VW, :], vext[:, jc, c0:c0 + VW], pt[:, 0:512],
                                         start=(jc == 0), stop=False)
                        nc.tensor.matmul(avO[0:VW, :], vext[:, jc, c1:c1 + VW], pt[:, 512:1024],
                                         start=(jc == 0), stop=False)
                        if filler is not None:
                            filler(b, p, h2, jc)
                    # leftover-pixel chunk closes the accumulation
                    nc.tensor.matmul(avE[0:VW, :],
                                     vext9s[b][32 * p:32 * p + NMEM, c0:c0 + VW],
                                     p9s[b][32 * p:32 * p + NMEM, h2, 0:512],
                                     start=False, stop=True,
                                     tile_position=(32 * p, 0))
                    nc.tensor.matmul(avO[0:VW, :],
                                     vext9s[b][32 * p:32 * p + NMEM, c1:c1 + VW],
                                     p9s[b][32 * p:32 * p + NMEM, h2, 512:1024],
                                     start=False, stop=True,
                                     tile_position=(32 * p, 0))
                    # denominators -> broadcast -> reciprocal -> attn
                    with tc.high_priority(offset=96):
                        dv = dvp.tile([128, 1024], F32R, tag="dv")
                        nc.vector.tensor_copy(out=dv[64:65, 0:512], in_=avE[64:65, :])
                        nc.vector.tensor_copy(out=dv[64:65, 512:1024], in_=avO[64:65, :])
                        bcpE = proj_ps.tile([128, 512], F32, tag="proj")
                        nc.tensor.matmul(bcpE[0:64, :], ones1[64:65, :], dv[64:65, 0:512],
                                         start=True, stop=True)
                        bcpO = proj_ps.tile([128, 512], F32, tag="proj")
                        nc.tensor.matmul(bcpO[0:64, :], ones1[64:65, :], dv[64:65, 512:1024],
                                         start=True, stop=True)
                        rcpE = rp.tile([64, 512], F32, tag="rcpE")
                        nc.vector.reciprocal_approx_fast(out=rcpE, in_=bcpE[0:64, :])
                        rcpO = rp.tile([64, 512], F32, tag="rcpO")
                        nc.vector.reciprocal_approx_fast(out=rcpO, in_=bcpO[0:64, :])
                        nc.vector.tensor_mul(
                            out=attns[b][0:64, p, h2 * 512:(h2 + 1) * 512],
                            in0=avE[0:64, :], in1=rcpE)
                        nc.vector.tensor_mul(
                            out=attns[b][64:128, p, h2 * 512:(h2 + 1) * 512],
                            in0=avO[0:64, :], in1=rcpO)

            # ---------------- schedule ----------------
            norm(0)
            norm(1)
            for p in range(NPAIR):
                for h2 in range(2):
                    kchain(0, p, h2)
                    qchain(0, p, h2)
            vchain(0, 7)
            for ic in range(7):
                vchain(0, ic)
            sim9(0)

            # filler queues: batch-1 qkv, its chunk-8 sims, batch-0 out proj
            fills = []
            for p in range(NPAIR):
                for h2 in range(2):
                    fills.append(lambda p=p, h2=h2: kchain(1, p, h2))
                    fills.append(lambda p=p, h2=h2: qchain(1, p, h2))
            fills.append(lambda: vchain(1, 7))
            fills.append(lambda: sim9(1))
            for ic in range(7):
                fills.append(lambda ic=ic: vchain(1, ic))
            for mc in range(CT):
                for h2 in range(2):
                    fills.append(lambda mc=mc, h2=h2: ochain(0, mc, h2))
            fcount = [0]

            def filler(b, p, h2, jc):
                unit = (b * NPAIR + p) * 16 + h2 * 8 + jc
                want = min(len(fills), (unit * 36) // 96 + 1)
                while fcount[0] < want:
                    fills[fcount[0]]()
                    fcount[0] += 1

            for b in range(PB):
                for p in range(NPAIR):
                    pair_attn(b, p, filler)
            while fcount[0] < len(fills):
                fills[fcount[0]]()
                fcount[0] += 1
            for mc in range(CT):
                for h2 in range(2):
                    ochain(1, mc, h2)
    nc.compile()
    return nc


_NC_CACHE = []


def _prep_inputs(x, gamma, mem_kv, w_qkv, w_out):
    b, c, hh, ww = x.shape
    n = hh * ww
    xs = x.reshape(b, c, n)

    g1 = gamma + 1.0
    wq = w_qkv[0:C] * (DH ** -0.5)
    wkv = w_qkv[C:]
    wqkv_eff = np.concatenate([wq, wkv], axis=0) * g1[None, :]
    wqkvt = np.ascontiguousarray(wqkv_eff.T)       # [c, 3c]
    wot = np.ascontiguousarray(w_out.T)            # [c, c]

    # memk: [128, heads, 4] - even head rows 0:64, odd head rows 64:128
    memk = np.zeros((128, HEADS, NMEM), np.float32)
    for h in range(HEADS):
        r0 = 64 * (h % 2)
        memk[r0:r0 + DH, h] = mem_kv[0, h].T
    # memv: [4, heads*(dh+1)] - v then the ones column
    memv = np.zeros((NMEM, HEADS * VW), np.float32)
    for h in range(HEADS):
        memv[:, h * VW:h * VW + DH] = mem_kv[1, h]
        memv[:, h * VW + DH] = 1.0

    import jax.numpy as jnp

    def bf(a):
        return np.asarray(jnp.asarray(a, dtype=jnp.bfloat16))

    return xs, bf(xs), bf(wqkvt), bf(wot), bf(memk), bf(memv)


def kernel(x, gamma, mem_kv, w_qkv, w_out, _trace=False):
    x = np.asarray(x, dtype=np.float32)
    gamma = np.asarray(gamma, dtype=np.float32)
    mem_kv = np.asarray(mem_kv, dtype=np.float32)
    w_qkv = np.asarray(w_qkv, dtype=np.float32)
    w_out = np.asarray(w_out, dtype=np.float32)

    b, c, hh, ww = x.shape
    xs, xbf, wqkvt, wot, memk, memv = _prep_inputs(x, gamma, mem_kv, w_qkv, w_out)

    if not _NC_CACHE:
        _NC_CACHE.append(_build())
    nc = _NC_CACHE[0]

    in_maps = []
    for core in range(NCORES):
        in_maps.append({
            "x": np.ascontiguousarray(xbf[core * PB:(core + 1) * PB]),
            "wqkv": wqkvt,
            "wot": wot,
            "memk": memk,
            "memv": memv,
        })
    res = run_bass_kernel_spmd(nc, in_maps, core_ids=list(range(NCORES)), trace=_trace)
    out = np.concatenate([res.results[core]["out"] for core in range(NCORES)], axis=0)
    kernel.last_result = res
    return out.reshape(b, c, hh, ww)


# revision 15
# speedup vs baseline: 1.1248x; 1.0653x over previous
"""Trainium2 Bass kernel for nn_Attention_7945689497706.

Distribution: data-parallel over batch, 2 batch elements per core, weights
replicated, no collectives.

v2 design (vs baseline):
  - Host folds (gamma+1) and dh^-0.5 into bf16 weights; x shipped bf16
    (halves input DMA, no on-chip weight prep).
  - Keys reordered per head as [pixels 0:1020 | mem(4) | pixels 1020:1024]:
    mem_kv rides chunk 7 rows 124:128 (DMA'd into place), the 4 leftover
    pixels form a tiny 9th chunk packed 4-pairs-per-psum via col tiling.
  - sim matmuls of a head PAIR run concurrently on disjoint PE row groups
    (even head rows 0:64, odd head rows 64:128) -> 2x sim throughput.
  - exp at [128, 1024] grain covering both heads of the pair.
  - softmax denominators via ones-column of v; per (pair, h2) a single K=2
    selector matmul broadcasts both heads' denominators, DVE reciprocal +
    psum-direct muls assemble attn.
  - Flat loop over 8 (batch, pair) units; qkv proj of batch 1 and out proj
    of batch 0 are spread as PE filler inside the loop.
"""

import numpy as np

import concourse.bass as bass
import concourse.mybir as mybir
import concourse.tile as tile
from concourse import bacc
from concourse.bass_utils import run_bass_kernel_spmd

F32 = mybir.dt.float32
F32R = mybir.dt.float32r
BF16 = mybir.dt.bfloat16
AF = mybir.ActivationFunctionType

NCORES = 8
B = 16
C = 512
N = 1024          # pixels = 32*32
HEADS = 8
DH = 64
NMEM = 4
PB = B // NCORES  # batch elements per core
CT = C // 128     # channel partition-tiles
NPAIR = 4         # head pairs
VW = DH + 1       # per head [v | ones]
KC = 1028         # keys per head: 1020 px + 4 mem + 4 px


def _build():
    nc = bacc.Bacc()
    x_ext = nc.declare_dram_parameter("x", [PB, C, N], BF16, isOutput=False)
    wqkv_ext = nc.declare_dram_parameter("wqkv", [C, 3 * C], BF16, isOutput=False)
    wot_ext = nc.declare_dram_parameter("wot", [C, C], BF16, isOutput=False)
    memk_ext = nc.declare_dram_parameter("memk", [128, HEADS, NMEM], BF16, isOutput=False)
    memv_ext = nc.declare_dram_parameter("memv", [NMEM, HEADS * VW], BF16, isOutput=False)
    out_ext = nc.declare_dram_parameter("out", [PB, C, N], F32, isOutput=True)

    with tile.TileContext(nc) as tc:
        with (
            tc.tile_pool(name="const", bufs=1) as const,
            tc.tile_pool(name="data", bufs=2) as data,
            tc.tile_pool(name="pp", bufs=6) as pp,
            tc.tile_pool(name="rp", bufs=2) as rp,
            tc.tile_pool(name="dvp", bufs=2) as dvp,
            tc.tile_pool(name="obp", bufs=2) as obp,
            tc.tile_pool(name="sim_ps", bufs=2, space="PSUM") as sim_ps,
            tc.tile_pool(name="av_ps", bufs=1, space="PSUM") as av_ps,
            tc.tile_pool(name="proj_ps", bufs=2, space="PSUM") as proj_ps,
        ):
            # ---------------- constants / big tiles ----------------
            wqkv = const.tile([128, CT, 3 * C], BF16, tag="wqkv")
            wo = const.tile([128, CT, C], BF16, tag="wo")
            ones128 = const.tile([128, 128], BF16, tag="ones128")
            ones1 = const.tile([128, 64], F32R, tag="ones1")

            def btiles(nm, shape, dtype):
                return [const.tile(shape, dtype, tag=f"{nm}{b}", name=f"{nm}{b}")
                        for b in range(PB)]

            xbs = btiles("xb", [128, CT, N], BF16)
            xns = btiles("xn", [128, CT, N], BF16)
            qTs = btiles("qT", [128, NPAIR, N], BF16)
            kTps = btiles("kTp", [128, HEADS, KC], BF16)
            vexts = btiles("vext", [128, 8, HEADS * VW], BF16)
            vext9s = btiles("v9", [128, HEADS * VW], BF16)
            p9s = btiles("p9", [128, 2, N], BF16)
            attns = btiles("attn", [128, CT, N], BF16)
            snorms = btiles("snorm", [128, N], F32)

            # ---------------- DMA plan ----------------
            # Few, large transfers (descriptor issue costs ~600ns of engine
            # time each). x on the sync queue; weights on the gpsimd queue so
            # the scalar engine stays free for activations.
            for t in range(CT):
                nc.sync.dma_start(out=xbs[0][:, t, :], in_=x_ext[0, t * 128:(t + 1) * 128, :])
            for t in range(CT):
                nc.gpsimd.dma_start(out=wqkv[:, t, :],
                                    in_=wqkv_ext[t * 128:(t + 1) * 128, :])
            for t in range(CT):
                nc.sync.dma_start(out=xbs[1][:, t, :], in_=x_ext[1, t * 128:(t + 1) * 128, :])
            for t in range(CT):
                nc.gpsimd.dma_start(out=wo[:, t, :], in_=wot_ext[t * 128:(t + 1) * 128, :])
            # mem K straight into place (kTp cols 1020:1024; disjoint from the
            # kchain writes). mem V is DMA'd inside vchain(b, 7) for ordering.
            for b in range(PB):
                nc.gpsimd.dma_start(out=kTps[b][:, :, 1020:1024], in_=memk_ext[:, :, :])

            nc.vector.memset(ones128, 1.0)
            nc.vector.memset(ones1.bitcast(F32), 1.0)
            # ones columns of vext, full-partition memsets (chunk-7 mem rows are
            # later overwritten by the memv DMA, which also carries 1.0 there)
            for b in range(PB):
                oc = vexts[b][:, :, :].rearrange("p j (h c) -> p j h c", c=VW)[:, :, :, DH:DH + 1]
                nc.gpsimd.memset(oc, 1.0)

            # ---------------- norm ----------------
            def norm(b):
                xb = xbs[b]
                xsq = data.tile([128, CT, N], BF16, tag="xsq", bufs=1)
                for t in range(CT):
                    nc.vector.tensor_mul(out=xsq[:, t, :], in0=xb[:, t, :], in1=xb[:, t, :])
                ss = sim_ps.tile([128, N], F32, tag="sim")
                for h2 in range(2):
                    for t in range(CT):
                        nc.tensor.matmul(ss[:, h2 * 512:(h2 + 1) * 512], ones128,
                                         xsq[:, t, h2 * 512:(h2 + 1) * 512],
                                         start=(t == 0), stop=(t == CT - 1))
                sroot = data.tile([128, N], F32, tag="sroot")
                nc.scalar.activation(out=sroot, in_=ss, func=AF.Sqrt, scale=1.0 / C)
                nc.vector.reciprocal_approx_fast(out=snorms[b], in_=sroot)
                for t in range(CT):
                    nc.vector.tensor_mul(out=xns[b][:, t, :], in0=xb[:, t, :], in1=snorms[b])

            # ---------------- projection chains ----------------
            def qchain(b, p, h2):
                ps = proj_ps.tile([128, 512], F32, tag="proj")
                for t in range(CT):
                    nc.tensor.matmul(ps, wqkv[:, t, 128 * p:128 * (p + 1)],
                                     xns[b][:, t, h2 * 512:(h2 + 1) * 512],
                                     start=(t == 0), stop=(t == CT - 1))
                nc.vector.tensor_copy(out=qTs[b][:, p, h2 * 512:(h2 + 1) * 512], in_=ps)

            def kchain(b, p, h2):
                ps = proj_ps.tile([128, 512], F32, tag="proj")
                for t in range(CT):
                    nc.tensor.matmul(ps, wqkv[:, t, C + 128 * p:C + 128 * (p + 1)],
                                     xns[b][:, t, h2 * 512:(h2 + 1) * 512],
                                     start=(t == 0), stop=(t == CT - 1))
                # kTp cols = pixel index for px<1020, 1024+ for the leftover 4
                kTp = kTps[b]
                h0, h1 = 2 * p, 2 * p + 1
                if h2 == 0:
                    nc.vector.tensor_copy(out=kTp[0:64, h0, 0:512], in_=ps[0:64, :])
                    nc.vector.tensor_copy(out=kTp[64:128, h1, 0:512], in_=ps[64:128, :])
                else:
                    nc.vector.tensor_copy(out=kTp[0:64, h0, 512:1020], in_=ps[0:64, 0:508])
                    nc.vector.tensor_copy(out=kTp[64:128, h1, 512:1020], in_=ps[64:128, 0:508])
                    nc.vector.tensor_copy(out=kTp[0:64, h0, 1024:1028], in_=ps[0:64, 508:512])
                    nc.vector.tensor_copy(out=kTp[64:128, h1, 1024:1028], in_=ps[64:128, 508:512])

            def vchain(b, ic):
                ps = proj_ps.tile([128, 512], F32, tag="proj")
                for t in range(CT):
                    nc.tensor.matmul(ps, xns[b][:, t, ic * 128:(ic + 1) * 128],
                                     wqkv[:, t, 2 * C:3 * C],
                                     start=(t == 0), stop=(t == CT - 1))
                ps_h = ps[:, :].rearrange("p (h c) -> p h c", c=DH)
                vdst = vexts[b][:, ic, :].rearrange("p (h c) -> p h c", c=VW)[:, :, 0:DH]
                nc.vector.tensor_copy(out=vdst, in_=ps_h)
                if ic == 7:
                    # rows 124:128 now hold the leftover-pixel v (+ones col from
                    # the memset): relocate each pair's 130-col block to its
                    # 32p row slot in vext9, then overwrite rows 124:128 with
                    # mem V (the DMA also carries its ones).
                    for p in range(NPAIR):
                        nc.sync.dma_start(
                            out=vext9s[b][32 * p:32 * p + NMEM, 2 * p * VW:(2 * p + 2) * VW],
                            in_=vexts[b][124:128, 7, 2 * p * VW:(2 * p + 2) * VW])
                    nc.sync.dma_start(out=vexts[b][124:128, 7, :], in_=memv_ext[:, :])

            def ochain(b, mc, h2):
                ps = proj_ps.tile([128, 512], F32, tag="proj")
                for t in range(CT):
                    nc.tensor.matmul(ps, wo[:, t, mc * 128:(mc + 1) * 128],
                                     attns[b][:, t, h2 * 512:(h2 + 1) * 512],
                                     start=(t == 0), stop=(t == CT - 1))
                ob = obp.tile([128, 512], F32, tag="ob")
                nc.vector.tensor_copy(out=ob, in_=ps)
                nc.sync.dma_start(
                    out=out_ext[b, mc * 128:(mc + 1) * 128, h2 * 512:(h2 + 1) * 512],
                    in_=ob)

            # ---------------- chunk-8 (leftover 4 pixels), all pairs packed ----
            def sim9(b):
                kTp, qT = kTps[b], qTs[b]
                for h2 in range(2):
                    st9e = proj_ps.tile([128, 512], F32, tag="proj")
                    for p in range(NPAIR):
                        nc.tensor.matmul(st9e[32 * p:32 * p + NMEM, :],
                                         kTp[0:64, 2 * p, 1024:1028],
                                         qT[0:64, p, h2 * 512:(h2 + 1) * 512],
                                         start=True, stop=True,
                                         tile_position=(0, 32 * p))
                    nc.scalar.activation(out=p9s[b][:, h2, 0:512], in_=st9e, func=AF.Exp)
                    st9o = proj_ps.tile([128, 512], F32, tag="proj")
                    for p in range(NPAIR):
                        nc.tensor.matmul(st9o[32 * p:32 * p + NMEM, :],
                                         kTp[64:128, 2 * p + 1, 1024:1028],
                                         qT[64:128, p, h2 * 512:(h2 + 1) * 512],
                                         start=True, stop=True,
                                         tile_position=(64, 32 * p))
                    nc.scalar.activation(out=p9s[b][:, h2, 512:1024], in_=st9o, func=AF.Exp)

            # ---------------- one attention unit: (batch, pair, h2) ----------
            # Software-pipelined emission: sims/exps run 2 chunks ahead of the
            # av accumulations, and the av tail + normalization drain of unit
            # u-1 is emitted after unit u's first two sims, so the ACT exp
            # stream never waits on the av/drain dependency chain.
            pending_tail = [None]

            def attn_unit(b, p, h2, filler):
                kTp, qT, vext = kTps[b], qTs[b], vexts[b]
                h0, h1 = 2 * p, 2 * p + 1
                c0, c1 = h0 * VW, h1 * VW
                avE = av_ps.tile([128, 512], F32, tag="avE")
                avO = av_ps.tile([128, 512], F32, tag="avO")
                pts = {}

                def av(jc, stop):
                    nc.tensor.matmul(avE[0:VW, :], vext[:, jc, c0:c0 + VW],
                                     pts[jc][:, 0:512], start=(jc == 0), stop=stop)
                    nc.tensor.matmul(avO[0:VW, :], vext[:, jc, c1:c1 + VW],
                                     pts[jc][:, 512:1024], start=(jc == 0), stop=stop)

                for jc in range(8):
                    st = sim_ps.tile([128, N], F32, tag="sim")
                    nc.tensor.matmul(st[:, 0:512],
                                     kTp[0:64, h0, jc * 128:(jc + 1) * 128],
                                     qT[0:64, p, h2 * 512:(h2 + 1) * 512],
                                     start=True, stop=True)
                    nc.tensor.matmul(st[:, 512:1024],
                                     kTp[64:128, h1, jc * 128:(jc + 1) * 128],
                                     qT[64:128, p, h2 * 512:(h2 + 1) * 512],
                                     start=True, stop=True)
                    pt = pp.tile([128, N], BF16, tag="p")
                    nc.scalar.activation(out=pt, in_=st, func=AF.Exp)
                    pts[jc] = pt
                    if jc == 1 and pending_tail[0] is not None:
                        pending_tail[0]()
                        pending_tail[0] = None
                    if jc >= 2:
                        av(jc - 2, stop=False)
                    if filler is not None:
                        filler(b, p, h2, jc)

                def tail():
                    av(6, stop=False)
                    av(7, stop=False)
                    nc.tensor.matmul(avE[0:VW, :],
                                     vext9s[b][32 * p:32 * p + NMEM, c0:c0 + VW],
                                     p9s[b][32 * p:32 * p + NMEM, h2, 0:512],
                                     start=False, stop=True,
                                     tile_position=(32 * p, 0))
                    nc.tensor.matmul(avO[0:VW, :],
                                     vext9s[b][32 * p:32 * p + NMEM, c1:c1 + VW],
                                     p9s[b][32 * p:32 * p + NMEM, h2, 512:1024],
                                     start=False, stop=True,
                                     tile_position=(32 * p, 0))
                    dv = dvp.tile([128, 1024], F32R, tag="dv")
                    nc.vector.tensor_copy(out=dv[64:65, 0:512], in_=avE[64:65, :])
                    nc.vector.tensor_copy(out=dv[64:65, 512:1024], in_=avO[64:65, :])
                    bcpE = proj_ps.tile([128, 512], F32, tag="proj")
                    nc.tensor.matmul(bcpE[0:64, :], ones1[64:65, :], dv[64:65, 0:512],
                                     start=True, stop=True)
                    bcpO = proj_ps.tile([128, 512], F32, tag="proj")
                    nc.tensor.matmul(bcpO[0:64, :], ones1[64:65, :], dv[64:65, 512:1024],
                                     start=True, stop=True)
                    rcpE = rp.tile([64, 512], F32, tag="rcpE")
                    nc.vector.reciprocal_approx_fast(out=rcpE, in_=bcpE[0:64, :])
                    rcpO = rp.tile([64, 512], F32, tag="rcpO")
                    nc.vector.reciprocal_approx_fast(out=rcpO, in_=bcpO[0:64, :])
                    nc.vector.tensor_mul(
                        out=attns[b][0:64, p, h2 * 512:(h2 + 1) * 512],
                        in0=avE[0:64, :], in1=rcpE)
                    nc.vector.tensor_mul(
                        out=attns[b][64:128, p, h2 * 512:(h2 + 1) * 512],
                        in0=avO[0:64, :], in1=rcpO)

                pending_tail[0] = tail

            # ---------------- schedule ----------------
            norm(0)
            norm(1)
            for p in range(NPAIR):
                for h2 in range(2):
                    kchain(0, p, h2)
                    qchain(0, p, h2)
            vchain(0, 7)
            for ic in range(7):
                vchain(0, ic)
            sim9(0)

            # filler queues: batch-1 qkv, its chunk-8 sims, batch-0 out proj
            fills = []
            for p in range(NPAIR):
                for h2 in range(2):
                    fills.append(lambda p=p, h2=h2: kchain(1, p, h2))
                    fills.append(lambda p=p, h2=h2: qchain(1, p, h2))
            fills.append(lambda: vchain(1, 7))
            fills.append(lambda: sim9(1))
            for ic in range(7):
                fills.append(lambda ic=ic: vchain(1, ic))
            for mc in range(CT):
                for h2 in range(2):
                    fills.append(lambda mc=mc, h2=h2: ochain(0, mc, h2))
            fcount = [0]

            def filler(b, p, h2, jc):
                unit = (b * NPAIR + p) * 16 + h2 * 8 + jc
                want = min(len(fills), (unit * 36) // 96 + 1)
                while fcount[0] < want:
                    fills[fcount[0]]()
                    fcount[0] += 1

            for b in range(PB):
                for p in range(NPAIR):
                    for h2 in range(2):
                        attn_unit(b, p, h2, filler)
            pending_tail[0]()
            pending_tail[0] = None
            while fcount[0] < len(fills):
                fills[fcount[0]]()
                fcount[0] += 1
            for mc in range(CT):
                for h2 in range(2):
                    ochain(1, mc, h2)
    nc.compile()
    return nc


_NC_CACHE = []


def _prep_inputs(x, gamma, mem_kv, w_qkv, w_out):
    b, c, hh, ww = x.shape
    n = hh * ww
    xs = x.reshape(b, c, n)

    g1 = gamma + 1.0
    wq = w_qkv[0:C] * (DH ** -0.5)
    wkv = w_qkv[C:]
    wqkv_eff = np.concatenate([wq, wkv], axis=0) * g1[None, :]
    wqkvt = np.ascontiguousarray(wqkv_eff.T)       # [c, 3c]
    wot = np.ascontiguousarray(w_out.T)            # [c, c]

    # memk: [128, heads, 4] - even head rows 0:64, odd head rows 64:128
    memk = np.zeros((128, HEADS, NMEM), np.float32)
    for h in range(HEADS):
        r0 = 64 * (h % 2)
        memk[r0:r0 + DH, h] = mem_kv[0, h].T
    # memv: [4, heads*(dh+1)] - v then the ones column
    memv = np.zeros((NMEM, HEADS * VW), np.float32)
    for h in range(HEADS):
        memv[:, h * VW:h * VW + DH] = mem_kv[1, h]
        memv[:, h * VW + DH] = 1.0

    import jax.numpy as jnp

    def bf(a):
        return np.asarray(jnp.asarray(a, dtype=jnp.bfloat16))

    return xs, bf(xs), bf(wqkvt), bf(wot), bf(memk), bf(memv)


def kernel(x, gamma, mem_kv, w_qkv, w_out, _trace=False):
    x = np.asarray(x, dtype=np.float32)
    gamma = np.asarray(gamma, dtype=np.float32)
    mem_kv = np.asarray(mem_kv, dtype=np.float32)
    w_qkv = np.asarray(w_qkv, dtype=np.float32)
    w_out = np.asarray(w_out, dtype=np.float32)

    b, c, hh, ww = x.shape
    xs, xbf, wqkvt, wot, memk, memv = _prep_inputs(x, gamma, mem_kv, w_qkv, w_out)

    if not _NC_CACHE:
        _NC_CACHE.append(_build())
    nc = _NC_CACHE[0]

    in_maps = []
    for core in range(NCORES):
        in_maps.append({
            "x": np.ascontiguousarray(xbf[core * PB:(core + 1) * PB]),
            "wqkv": wqkvt,
            "wot": wot,
            "memk": memk,
            "memv": memv,
        })
    res = run_bass_kernel_spmd(nc, in_maps, core_ids=list(range(NCORES)), trace=_trace)
    out = np.concatenate([res.results[core]["out"] for core in range(NCORES)], axis=0)
    kernel.last_result = res
    return out.reshape(b, c, hh, ww)
